# revision 10
# baseline (speedup 1.0000x reference)
"""EDAC layer kernel for Trainium2 (8 NeuronCores, batch-sharded SPMD).

Reference semantics (B=32, C=256, K=64, H=W=56; vulnerable_idx == arange(K)):
  valid(x, c)  = min_vals[c] <= x <= max_vals[c]
  channels >= K:  out = x if valid else 0
  channels <  K:  m = main, d = dup
      both valid  -> min(m, d)      (covers m == d too)
      only d      -> d
      only m      -> m
      neither     -> 0

v3 design, driven by measured per-op DVE/ScalarE costs and the per-op perf
mode table (scalar_tensor_tensor: always 1x; tensor_tensor: 2x with all-bf16
operands; tensor_scalar(2 scalars): up to 4x; Pool: dtype-cast tensor_copy):

  * All stores fp16 (write traffic halves; l2 error ~1e-3, gate is 2e-2).
  * Sentinel formulation per guarded input x with per-channel (mid, rad):
      ScalarE:  q  = Square(x - mid)           (fp32, sign-exact compare)
                r' = Relu(HUGE*q - HUGE*rad^2) (bf16; 0 iff valid else >=1e22)
      Pool:     xb = tensor_copy(x)            (fp32 -> bf16 value copy)
      DVE:      s  = xb + r'                   (tensor_tensor add, bf16 2x)
                out= fmod(min(s, THR), THR)    (tensor_scalar fused, 4x;
                                                exact identity for |s| < THR,
                                                exactly 0 for sentinels)
    Vulnerable tiles build s_m and s_d this way and insert min(s_m, s_d)
    (bf16 tensor_tensor, 2x) before the fmod threshold.
  * Two simple tiles (A1, C1 -- the late ones) use the sentinel path; the
    other four use the plain two-pass scalar_tensor_tensor compare-mult
    (fused compare+apply, 1x) to keep ScalarE's 12-pass chain inside the
    DMA window.  Engine budget: DVE ~53us, ScalarE ~43us, Pool ~21us,
    PE unused, DMA 22.6MB @ ~430GB/s = ~53us.
  * bf16 value rounding (sentinel-path outputs) adds ~2e-3 l2; compare
    decisions always happen against fp32 inputs so no boundary flips
    beyond O(ulp) shifts of the parabola test.
"""

import os
import sys

for _p in ("/opt/trn_rl_repo", os.path.expanduser("~/.axon_site/_ro/trn_rl_repo")):
    if os.path.isdir(_p) and _p not in sys.path:
        sys.path.insert(0, _p)

import numpy as np

import concourse.bass as bass
import concourse.bacc as bacc
import concourse.mybir as mybir
from concourse.tile import TileContext
from concourse.bass_utils import run_bass_kernel_spmd

F32 = mybir.dt.float32
F16 = mybir.dt.float16
BF16 = mybir.dt.bfloat16
OP = mybir.AluOpType
AF = mybir.ActivationFunctionType

B, C, K, H, W = 32, 256, 64, 56, 56
HW = H * W
NCORES = 8
BL = B // NCORES  # batches per core

HUGE = 1.0e30  # sentinel prescale: HUGE * (q - rad^2) >> THR for any
               # practically-representable positive margin
THR = 1.0e15   # valid values are <= ~10; invalid sentinels are >= ~1e22

# bounds table columns (per-partition scalars for each tile kind)
#   0..3  : lo            for tile kinds A, B, C, V
#   4..7  : hi            for tile kinds A, B, C, V
#   8..11 : -mid          for tile kinds A, B, C, V  (mid = (lo+hi)/2)
#   12..15: -HUGE*rad^2   for tile kinds A, B, C, V  (rad = (hi-lo)/2)
NBCOLS = 16


def build_bounds(min_vals: np.ndarray, max_vals: np.ndarray) -> np.ndarray:
    lo = np.asarray(min_vals, dtype=np.float64)
    hi = np.asarray(max_vals, dtype=np.float64)
    cols = np.zeros((128, NBCOLS), dtype=np.float64)
    interleave = lambda a, b: np.stack([a, b], axis=1).ravel()
    kinds = [
        np.arange(64, 192),                                   # A: ch 64..191
        interleave(np.arange(192, 256), np.arange(64, 128)),  # B (interleaved)
        np.arange(128, 256),                                  # C: ch 128..255
        np.repeat(np.arange(0, 64), 2),                       # V (interleaved)
    ]
    for j, idx in enumerate(kinds):
        cols[:, j] = lo[idx]
        cols[:, 4 + j] = hi[idx]
        mid = (lo[idx] + hi[idx]) / 2.0
        rad = (hi[idx] - lo[idx]) / 2.0
        cols[:, 8 + j] = -mid
        cols[:, 12 + j] = -(HUGE * rad * rad)
    return cols.astype(np.float32)


def build_nc(hw: int = HW) -> bass.Bass:
    nc = bacc.Bacc("TRN2", target_bir_lowering=False, debug=False)
    R = BL * C
    main = nc.dram_tensor("main", [R, hw], F32, kind="ExternalInput")
    dup = nc.dram_tensor("dup", [BL * K, hw], F32, kind="ExternalInput")
    bounds = nc.dram_tensor("bounds", [128, NBCOLS], F32, kind="ExternalInput")
    out = nc.dram_tensor("out", [R, hw], F16, kind="ExternalOutput")

    npairs = BL // 2

    # Per-pair DRAM views.
    main_p = main.ap().rearrange("(p x) w -> p x w", p=npairs)   # [p, 512, hw]
    out_p = out.ap().rearrange("(p x) w -> p x w", p=npairs)
    dup_p = dup.ap().rearrange("(p s c) w -> p c s w", p=npairs, s=2)

    def v_ap(t):   # [64, 2, hw]: ch 0..63 of batches b, b+1 interleaved
        return t.rearrange("(s g c) w -> g c s w", s=2, g=4)[0]

    def b_ap(t):   # [64, 2, hw]: ch 192..255 of b / ch 64..127 of b+1
        return t[192:384].rearrange("(s c) w -> c s w", s=3)[:, 0:3:2]

    APS = {
        0: lambda t: t[64:192],      # A
        1: b_ap,                     # B
        2: lambda t: t[384:512],     # C
    }

    with TileContext(nc) as tc:
        with (
            tc.tile_pool(name="bnd", bufs=1) as bpool,
            tc.tile_pool(name="pm", bufs=6) as pm,
            tc.tile_pool(name="pv", bufs=2) as pv,
            tc.tile_pool(name="pd", bufs=2) as pd,
            tc.tile_pool(name="pq", bufs=1) as pq,
            tc.tile_pool(name="pr", bufs=4) as pr,
            tc.tile_pool(name="pb", bufs=3) as pb,
            tc.tile_pool(name="po", bufs=4) as po,
        ):
            bt = bpool.tile([128, NBCOLS], F32)
            nc.sync.dma_start(out=bt[:], in_=bounds[:])

            def lo_ap(j):
                return bt[:, j:j + 1]

            def hi_ap(j):
                return bt[:, 4 + j:5 + j]

            def nmid_ap(j):
                return bt[:, 8 + j:9 + j]

            def nhrad2_ap(j):
                return bt[:, 12 + j:13 + j]

            vd = [None] * npairs
            abc = [[None] * 3 for _ in range(npairs)]
            half = hw // 2

            def load_vd(p):
                mv = pv.tile([128, hw], F32, tag="mv")
                nc.sync.dma_start(out=mv[:], in_=v_ap(main_p[p]))
                dv = pd.tile([128, hw], F32, tag="dv")
                nc.sync.dma_start(out=dv[:], in_=dup_p[p])
                vd[p] = (mv, dv)

            def load_simple(p, kind, head=False):
                mt = pm.tile([128, hw], F32, tag="mt")
                abc[p][kind] = mt
                src_ap = APS[kind](main_p[p])
                if head:  # two half DMAs (returns the second so the caller
                    # can interleave other loads between them)
                    nc.sync.dma_start(out=mt[:, 0:half], in_=src_ap[..., 0:half])
                    return lambda: nc.sync.dma_start(
                        out=mt[:, half:hw], in_=src_ap[..., half:hw])
                nc.sync.dma_start(out=mt[:], in_=src_ap)
                return None

            # Load order: A0 halves first (DVE start), V0/D0 early (26us
            # ScalarE chain paces the vuln path), then B0, V1/D1, A1 before
            # C0/B1 (A1 is on the sentinel path: ScalarE needs it by ~34us).
            a0t = load_simple(0, 0, head=True)   # A0 first half
            mv0 = pv.tile([128, hw], F32, tag="mv", name="mv0")
            nc.sync.dma_start(out=mv0[:], in_=v_ap(main_p[0]))
            a0t()                                # A0 second half
            dv0 = pd.tile([128, hw], F32, tag="dv", name="dv0")
            nc.sync.dma_start(out=dv0[:], in_=dup_p[0])
            vd[0] = (mv0, dv0)
            load_simple(0, 1)                    # B0
            load_vd(1)                           # V1, D1
            load_simple(1, 0)                    # A1 (sentinel path)
            load_simple(0, 2)                    # C0
            load_simple(1, 1)                    # B1
            load_simple(1, 2)                    # C1 (sentinel path)

            # ScalarE compare chain: q = (x-mid)^2 in fp32 (sign-exact),
            # r' = relu(HUGE*q - HUGE*rad^2) in bf16 (0 iff valid, else
            # >= ~1e22).  Pool casts the value stream to bf16.
            def make_sent(x, j, name):
                q = pq.tile([128, hw], F32, tag="q", name=f"q{name}")
                nc.scalar.activation(q[:], x[:], AF.Square, bias=nmid_ap(j))
                r = pr.tile([128, hw], BF16, tag="rl", name=f"r{name}")
                nc.scalar.activation(r[:], q[:], AF.Relu, scale=HUGE,
                                     bias=nhrad2_ap(j))
                xb = pb.tile([128, hw], BF16, tag="xb", name=f"xb{name}")
                nc.gpsimd.tensor_copy(out=xb[:], in_=x[:])
                return r, xb

            sent = []
            for p in range(npairs):
                mv, dv = vd[p]
                sent.append((make_sent(mv, 3, f"m{p}"),
                             make_sent(dv, 3, f"d{p}")))

            def thresh(ot, s, msk):
                # msk = (s < THR) in bf16 (exact 0/1), out = msk * s in fp16.
                # ts is 4x-capable on 16-bit SBUF operands, tt is 2x.
                nc.vector.tensor_scalar(out=msk[:], in0=s[:], scalar1=THR,
                                        scalar2=0.0, op0=OP.is_lt,
                                        op1=OP.bypass)
                nc.vector.tensor_tensor(out=ot[:], in0=msk[:], in1=s[:],
                                        op=OP.mult)

            def do_simple(p, kind, split=False):
                """Plain simple path on DVE: two fused compare-mult stt
                passes (1x), the second writing fp16."""
                mt = abc[p][kind]
                ot = po.tile([128, hw], F16, tag="ot")
                dst = APS[kind](out_p[p])
                stt = nc.vector.scalar_tensor_tensor
                halves = (slice(0, half), slice(half, hw)) if split \
                    else (slice(0, hw),)
                for cs in halves:
                    stt(out=mt[:, cs], in0=mt[:, cs], scalar=lo_ap(kind),
                        in1=mt[:, cs], op0=OP.is_ge, op1=OP.mult)
                    stt(out=ot[:, cs], in0=mt[:, cs], scalar=hi_ap(kind),
                        in1=mt[:, cs], op0=OP.is_le, op1=OP.mult)
                    nc.sync.dma_start(out=dst[..., cs], in_=ot[:, cs])

            def do_simple_sent(p, kind):
                """Sentinel-path simple tile: ScalarE compares, Pool value
                cast, DVE mask = (r' == 0) (ts, 4x) + apply (tt, 2x)."""
                mt = abc[p][kind]
                r, xb = make_sent(mt, kind, f"s{p}{kind}")
                ot = po.tile([128, hw], F16, tag="ot")
                nc.vector.tensor_scalar(out=r[:], in0=r[:], scalar1=0.0,
                                        scalar2=0.0, op0=OP.is_equal,
                                        op1=OP.bypass)
                nc.vector.tensor_tensor(out=ot[:], in0=r[:], in1=xb[:],
                                        op=OP.mult)
                nc.sync.dma_start(out=APS[kind](out_p[p])[...], in_=ot[:])

            def do_vuln(p):
                """Vuln tile: s_m = bf16(m) + r'_m, s_d = bf16(d) + r'_d,
                u = min(s_m, s_d) (all bf16 tensor_tensor, 2x), then
                mask = (u < THR) (ts) and out = mask * u (tt)."""
                (rm, mb), (rd, db) = sent[p]
                ot = po.tile([128, hw], F16, tag="ot", name=f"vot{p}")
                tt = nc.vector.tensor_tensor
                tt(out=rm[:], in0=mb[:], in1=rm[:], op=OP.add)
                tt(out=rd[:], in0=db[:], in1=rd[:], op=OP.add)
                tt(out=rd[:], in0=rm[:], in1=rd[:], op=OP.min)
                thresh(ot, rd, rm)
                nc.sync.dma_start(out=v_ap(out_p[p])[...], in_=ot[:])

            # DVE queue order: plain tiles early (paced by loads), vuln and
            # sentinel tiles interleaved where their ScalarE/Pool inputs are
            # ready, sentinel applies last (cheap, ~3us each).
            do_simple(0, 0, split=True)     # A0
            do_simple(0, 1, split=True)     # B0
            do_vuln(0)                      # V0
            do_simple(0, 2)                 # C0
            do_simple(1, 1)                 # B1
            do_simple_sent(1, 0)            # A1
            do_vuln(1)                      # V1
            do_simple_sent(1, 2)            # C1
    return nc


_NC_CACHE: dict = {}


def _get_nc(hw: int) -> bass.Bass:
    if hw not in _NC_CACHE:
        nc = build_nc(hw)
        nc.finalize()  # Bacc.finalize runs compile() (register allocation etc.)
        _NC_CACHE[hw] = nc
    return _NC_CACHE[hw]


def kernel(main_out, dup_out, min_vals, max_vals, vulnerable_idx):
    return _run(main_out, dup_out, min_vals, max_vals, vulnerable_idx)[0]


def _run(main_out, dup_out, min_vals, max_vals, vulnerable_idx, **spmd_kwargs):
    main_out = np.asarray(main_out)
    dup_out = np.asarray(dup_out)
    min_vals = np.asarray(min_vals)
    max_vals = np.asarray(max_vals)
    vidx = np.asarray(vulnerable_idx).ravel()

    # Device kernel assumes vulnerable channels are 0..K-1. If not, permute
    # channels host-side so they are, and invert on the way out.
    perm = None
    if not np.array_equal(vidx, np.arange(K)):
        assert len(np.unique(vidx)) == K, "duplicate vulnerable_idx unsupported"
        rest = np.setdiff1d(np.arange(C), vidx)
        perm = np.concatenate([vidx, rest])
        main_out = main_out[:, perm]
        min_vals = min_vals[perm]
        max_vals = max_vals[perm]

    mo = np.ascontiguousarray(main_out, dtype=np.float32).reshape(B, C, HW)
    du = np.ascontiguousarray(dup_out, dtype=np.float32).reshape(B, K, HW)
    bounds = build_bounds(min_vals, max_vals)

    in_maps = []
    for k in range(NCORES):
        in_maps.append({
            "main": mo[BL * k:BL * (k + 1)].reshape(BL * C, HW),
            "dup": du[BL * k:BL * (k + 1)].reshape(BL * K, HW),
            "bounds": bounds,
        })

    nc = _get_nc(HW)
    res = run_bass_kernel_spmd(nc, in_maps, list(range(NCORES)), **spmd_kwargs)
    out = np.concatenate(
        [r["out"].astype(np.float32).reshape(BL, C, H, W) for r in res.results],
        axis=0)

    if perm is not None:
        inv = np.empty(C, dtype=np.int64)
        inv[perm] = np.arange(C)
        out = out[:, inv]
    return out, res


# revision 11
# speedup vs baseline: 1.4139x; 1.4139x over previous
"""EDAC layer kernel for Trainium2 (8 NeuronCores, batch-sharded SPMD).

Reference semantics (B=32, C=256, K=64, H=W=56; vulnerable_idx == arange(K)):
  valid(x, c)  = min_vals[c] <= x <= max_vals[c]
  channels >= K:  out = x if valid else 0
  channels <  K:  m = main, d = dup
      both valid  -> min(m, d)      (covers m == d too)
      only d      -> d
      only m      -> m
      neither     -> 0

v3 design, driven by measured per-op DVE/ScalarE costs and the per-op perf
mode table (scalar_tensor_tensor: always 1x; tensor_tensor: 2x with all-bf16
operands; tensor_scalar(2 scalars): up to 4x; Pool: dtype-cast tensor_copy):

  * All stores fp16 (write traffic halves; l2 error ~1e-3, gate is 2e-2).
  * Sentinel formulation per guarded input x with per-channel (mid, rad):
      ScalarE:  q  = Square(x - mid)           (fp32, sign-exact compare)
                r' = Relu(HUGE*q - HUGE*rad^2) (bf16; 0 iff valid else >=1e22)
      Pool:     xb = tensor_copy(x)            (fp32 -> bf16 value copy)
      DVE:      s  = xb + r'                   (tensor_tensor add, bf16 2x)
                out= fmod(min(s, THR), THR)    (tensor_scalar fused, 4x;
                                                exact identity for |s| < THR,
                                                exactly 0 for sentinels)
    Vulnerable tiles build s_m and s_d this way and insert min(s_m, s_d)
    (bf16 tensor_tensor, 2x) before the fmod threshold.
  * Two simple tiles (A1, C1 -- the late ones) use the sentinel path; the
    other four use the plain two-pass scalar_tensor_tensor compare-mult
    (fused compare+apply, 1x) to keep ScalarE's 12-pass chain inside the
    DMA window.  Engine budget: DVE ~53us, ScalarE ~43us, Pool ~21us,
    PE unused, DMA 22.6MB @ ~430GB/s = ~53us.
  * bf16 value rounding (sentinel-path outputs) adds ~2e-3 l2; compare
    decisions always happen against fp32 inputs so no boundary flips
    beyond O(ulp) shifts of the parabola test.
"""

import os
import sys

for _p in ("/opt/trn_rl_repo", os.path.expanduser("~/.axon_site/_ro/trn_rl_repo")):
    if os.path.isdir(_p) and _p not in sys.path:
        sys.path.insert(0, _p)

import numpy as np

import concourse.bass as bass
import concourse.bacc as bacc
import concourse.mybir as mybir
from concourse.tile import TileContext
from concourse.bass_utils import run_bass_kernel_spmd

F32 = mybir.dt.float32
F16 = mybir.dt.float16
BF16 = mybir.dt.bfloat16
OP = mybir.AluOpType
AF = mybir.ActivationFunctionType

B, C, K, H, W = 32, 256, 64, 56, 56
HW = H * W
NCORES = 8
BL = B // NCORES  # batches per core

HUGE = 1.0e30  # sentinel prescale: HUGE * (q - rad^2) >> THR for any
               # practically-representable positive margin
THR = 1.0e15   # valid values are <= ~10; invalid sentinels are >= ~1e22

# bounds table columns (per-partition scalars for each tile kind)
#   0..3  : lo            for tile kinds A, B, C, V
#   4..7  : hi            for tile kinds A, B, C, V
#   8..11 : -mid          for tile kinds A, B, C, V  (mid = (lo+hi)/2)
#   12..15: -HUGE*rad^2   for tile kinds A, B, C, V  (rad = (hi-lo)/2)
NBCOLS = 16


def build_bounds(min_vals: np.ndarray, max_vals: np.ndarray) -> np.ndarray:
    lo = np.asarray(min_vals, dtype=np.float64)
    hi = np.asarray(max_vals, dtype=np.float64)
    cols = np.zeros((128, NBCOLS), dtype=np.float64)
    interleave = lambda a, b: np.stack([a, b], axis=1).ravel()
    kinds = [
        np.arange(64, 192),                                   # A: ch 64..191
        interleave(np.arange(192, 256), np.arange(64, 128)),  # B (interleaved)
        np.arange(128, 256),                                  # C: ch 128..255
        np.repeat(np.arange(0, 64), 2),                       # V (interleaved)
    ]
    for j, idx in enumerate(kinds):
        cols[:, j] = lo[idx]
        cols[:, 4 + j] = hi[idx]
        mid = (lo[idx] + hi[idx]) / 2.0
        rad = (hi[idx] - lo[idx]) / 2.0
        cols[:, 8 + j] = -mid
        cols[:, 12 + j] = -(HUGE * rad * rad)
    return cols.astype(np.float32)


def build_nc(hw: int = HW) -> bass.Bass:
    nc = bacc.Bacc("TRN2", target_bir_lowering=False, debug=False)
    R = BL * C
    main = nc.dram_tensor("main", [R, hw], F32, kind="ExternalInput")
    dup = nc.dram_tensor("dup", [BL * K, hw], F32, kind="ExternalInput")
    bounds = nc.dram_tensor("bounds", [128, NBCOLS], F32, kind="ExternalInput")
    out = nc.dram_tensor("out", [R, hw], F16, kind="ExternalOutput")

    npairs = BL // 2

    # Per-pair DRAM views.
    main_p = main.ap().rearrange("(p x) w -> p x w", p=npairs)   # [p, 512, hw]
    out_p = out.ap().rearrange("(p x) w -> p x w", p=npairs)
    dup_p = dup.ap().rearrange("(p s c) w -> p c s w", p=npairs, s=2)

    def v_ap(t):   # [64, 2, hw]: ch 0..63 of batches b, b+1 interleaved
        return t.rearrange("(s g c) w -> g c s w", s=2, g=4)[0]

    def b_ap(t):   # [64, 2, hw]: ch 192..255 of b / ch 64..127 of b+1
        return t[192:384].rearrange("(s c) w -> c s w", s=3)[:, 0:3:2]

    APS = {
        0: lambda t: t[64:192],      # A
        1: b_ap,                     # B
        2: lambda t: t[384:512],     # C
    }

    with TileContext(nc) as tc:
        with (
            tc.tile_pool(name="bnd", bufs=1) as bpool,
            tc.tile_pool(name="pm", bufs=6) as pm,
            tc.tile_pool(name="pv", bufs=2) as pv,
            tc.tile_pool(name="pd", bufs=2) as pd,
            tc.tile_pool(name="pq", bufs=1) as pq,
            tc.tile_pool(name="pr", bufs=4) as pr,
            tc.tile_pool(name="pb", bufs=3) as pb,
            tc.tile_pool(name="po", bufs=4) as po,
        ):
            bt = bpool.tile([128, NBCOLS], F32)
            nc.sync.dma_start(out=bt[:], in_=bounds[:])

            def lo_ap(j):
                return bt[:, j:j + 1]

            def hi_ap(j):
                return bt[:, 4 + j:5 + j]

            def nmid_ap(j):
                return bt[:, 8 + j:9 + j]

            def nhrad2_ap(j):
                return bt[:, 12 + j:13 + j]

            vd = [None] * npairs
            abc = [[None] * 3 for _ in range(npairs)]
            half = hw // 2

            def load_vd(p):
                mv = pv.tile([128, hw], F32, tag="mv")
                nc.sync.dma_start(out=mv[:], in_=v_ap(main_p[p]))
                dv = pd.tile([128, hw], F32, tag="dv")
                nc.sync.dma_start(out=dv[:], in_=dup_p[p])
                vd[p] = (mv, dv)

            def load_simple(p, kind, head=False):
                mt = pm.tile([128, hw], F32, tag="mt")
                abc[p][kind] = mt
                src_ap = APS[kind](main_p[p])
                if head:  # two half DMAs (returns the second so the caller
                    # can interleave other loads between them)
                    nc.sync.dma_start(out=mt[:, 0:half], in_=src_ap[..., 0:half])
                    return lambda: nc.sync.dma_start(
                        out=mt[:, half:hw], in_=src_ap[..., half:hw])
                nc.sync.dma_start(out=mt[:], in_=src_ap)
                return None

            # Load order: A0 halves first (DVE start), V0/D0 early (26us
            # ScalarE chain paces the vuln path), then B0, V1/D1, A1 before
            # C0/B1 (A1 is on the sentinel path: ScalarE needs it by ~34us).
            a0t = load_simple(0, 0, head=True)   # A0 first half
            mv0 = pv.tile([128, hw], F32, tag="mv", name="mv0")
            nc.sync.dma_start(out=mv0[:], in_=v_ap(main_p[0]))
            a0t()                                # A0 second half
            dv0 = pd.tile([128, hw], F32, tag="dv", name="dv0")
            nc.sync.dma_start(out=dv0[:], in_=dup_p[0])
            vd[0] = (mv0, dv0)
            load_simple(0, 1)                    # B0
            load_vd(1)                           # V1, D1
            load_simple(0, 2)                    # C0
            load_simple(1, 0)                    # A1 (sentinel path)
            load_simple(1, 1)                    # B1
            load_simple(1, 2)                    # C1 (sentinel path)

            # ScalarE compare chain: q = (x-mid)^2 in fp32 (sign-exact),
            # r' = relu(HUGE*q - HUGE*rad^2) in bf16 (0 iff valid, else
            # >= ~1e22).  Pool casts the value stream to bf16.
            def make_sent(x, j, name):
                q = pq.tile([128, hw], F32, tag="q", name=f"q{name}")
                nc.scalar.activation(q[:], x[:], AF.Square, bias=nmid_ap(j))
                r = pr.tile([128, hw], BF16, tag="rl", name=f"r{name}")
                nc.scalar.activation(r[:], q[:], AF.Relu, scale=HUGE,
                                     bias=nhrad2_ap(j))
                xb = pb.tile([128, hw], BF16, tag="xb", name=f"xb{name}")
                nc.gpsimd.dma_start(out=xb[:], in_=x[:])  # SWDGE cast-DMA
                return r, xb

            sent = []
            for p in range(npairs):
                mv, dv = vd[p]
                sent.append((make_sent(mv, 3, f"m{p}"),
                             make_sent(dv, 3, f"d{p}")))

            def thresh(ot, s, msk):
                # msk = (s < THR) in bf16 (exact 0/1), out = msk * s in fp16.
                # ts is 4x-capable on 16-bit SBUF operands, tt is 2x.
                nc.vector.tensor_scalar(out=msk[:], in0=s[:], scalar1=THR,
                                        scalar2=0.0, op0=OP.is_lt,
                                        op1=OP.bypass)
                nc.vector.tensor_tensor(out=ot[:], in0=msk[:], in1=s[:],
                                        op=OP.mult)

            def do_simple(p, kind, split=False):
                """Plain simple path on DVE: two fused compare-mult stt
                passes (1x), the second writing fp16."""
                mt = abc[p][kind]
                ot = po.tile([128, hw], F16, tag="ot")
                dst = APS[kind](out_p[p])
                stt = nc.vector.scalar_tensor_tensor
                halves = (slice(0, half), slice(half, hw)) if split \
                    else (slice(0, hw),)
                for cs in halves:
                    stt(out=mt[:, cs], in0=mt[:, cs], scalar=lo_ap(kind),
                        in1=mt[:, cs], op0=OP.is_ge, op1=OP.mult)
                    stt(out=ot[:, cs], in0=mt[:, cs], scalar=hi_ap(kind),
                        in1=mt[:, cs], op0=OP.is_le, op1=OP.mult)
                    nc.sync.dma_start(out=dst[..., cs], in_=ot[:, cs])

            def do_simple_sent(p, kind):
                """Sentinel-path simple tile: ScalarE compares, Pool value
                cast, DVE mask = (r' == 0) (ts, 4x) + apply (tt, 2x)."""
                mt = abc[p][kind]
                r, xb = make_sent(mt, kind, f"s{p}{kind}")
                ot = po.tile([128, hw], F16, tag="ot")
                nc.vector.tensor_scalar(out=r[:], in0=r[:], scalar1=0.0,
                                        scalar2=0.0, op0=OP.is_equal,
                                        op1=OP.bypass)
                nc.vector.tensor_tensor(out=ot[:], in0=r[:], in1=xb[:],
                                        op=OP.mult)
                nc.sync.dma_start(out=APS[kind](out_p[p])[...], in_=ot[:])

            def do_vuln(p):
                """Vuln tile: s_m = bf16(m) + r'_m, s_d = bf16(d) + r'_d,
                u = min(s_m, s_d) (all bf16 tensor_tensor, 2x), then
                mask = (u < THR) (ts) and out = mask * u (tt)."""
                (rm, mb), (rd, db) = sent[p]
                ot = po.tile([128, hw], F16, tag="ot", name=f"vot{p}")
                tt = nc.vector.tensor_tensor
                tt(out=rm[:], in0=mb[:], in1=rm[:], op=OP.add)
                tt(out=rd[:], in0=db[:], in1=rd[:], op=OP.add)
                tt(out=rd[:], in0=rm[:], in1=rd[:], op=OP.min)
                thresh(ot, rd, rm)
                nc.sync.dma_start(out=v_ap(out_p[p])[...], in_=ot[:])

            # DVE queue order: plain tiles early (paced by loads), vuln and
            # sentinel tiles interleaved where their ScalarE/Pool inputs are
            # ready, sentinel applies last (cheap, ~3us each).
            do_simple(0, 0, split=True)     # A0
            do_simple(0, 1, split=True)     # B0
            do_vuln(0)                      # V0
            do_simple(0, 2)                 # C0
            do_simple(1, 1)                 # B1
            do_simple_sent(1, 0)            # A1
            do_vuln(1)                      # V1
            do_simple_sent(1, 2)            # C1
    return nc


_NC_CACHE: dict = {}


def _get_nc(hw: int) -> bass.Bass:
    if hw not in _NC_CACHE:
        nc = build_nc(hw)
        nc.finalize()  # Bacc.finalize runs compile() (register allocation etc.)
        _NC_CACHE[hw] = nc
    return _NC_CACHE[hw]


def kernel(main_out, dup_out, min_vals, max_vals, vulnerable_idx):
    return _run(main_out, dup_out, min_vals, max_vals, vulnerable_idx)[0]


def _run(main_out, dup_out, min_vals, max_vals, vulnerable_idx, **spmd_kwargs):
    main_out = np.asarray(main_out)
    dup_out = np.asarray(dup_out)
    min_vals = np.asarray(min_vals)
    max_vals = np.asarray(max_vals)
    vidx = np.asarray(vulnerable_idx).ravel()

    # Device kernel assumes vulnerable channels are 0..K-1. If not, permute
    # channels host-side so they are, and invert on the way out.
    perm = None
    if not np.array_equal(vidx, np.arange(K)):
        assert len(np.unique(vidx)) == K, "duplicate vulnerable_idx unsupported"
        rest = np.setdiff1d(np.arange(C), vidx)
        perm = np.concatenate([vidx, rest])
        main_out = main_out[:, perm]
        min_vals = min_vals[perm]
        max_vals = max_vals[perm]

    mo = np.ascontiguousarray(main_out, dtype=np.float32).reshape(B, C, HW)
    du = np.ascontiguousarray(dup_out, dtype=np.float32).reshape(B, K, HW)
    bounds = build_bounds(min_vals, max_vals)

    in_maps = []
    for k in range(NCORES):
        in_maps.append({
            "main": mo[BL * k:BL * (k + 1)].reshape(BL * C, HW),
            "dup": du[BL * k:BL * (k + 1)].reshape(BL * K, HW),
            "bounds": bounds,
        })

    nc = _get_nc(HW)
    res = run_bass_kernel_spmd(nc, in_maps, list(range(NCORES)), **spmd_kwargs)
    out = np.concatenate(
        [r["out"].astype(np.float32).reshape(BL, C, H, W) for r in res.results],
        axis=0)

    if perm is not None:
        inv = np.empty(C, dtype=np.int64)
        inv[perm] = np.arange(C)
        out = out[:, inv]
    return out, res


# revision 12
# speedup vs baseline: 1.6367x; 1.1576x over previous
"""EDAC layer kernel for Trainium2 (8 NeuronCores, batch-sharded SPMD).

Reference semantics (B=32, C=256, K=64, H=W=56; vulnerable_idx == arange(K)):
  valid(x, c)  = min_vals[c] <= x <= max_vals[c]
  channels >= K:  out = x if valid else 0
  channels <  K:  m = main, d = dup
      both valid  -> min(m, d)      (covers m == d too)
      only d      -> d
      only m      -> m
      neither     -> 0

v3 design, driven by measured per-op DVE/ScalarE costs and the per-op perf
mode table (scalar_tensor_tensor: always 1x; tensor_tensor: 2x with all-bf16
operands; tensor_scalar(2 scalars): up to 4x; Pool: dtype-cast tensor_copy):

  * All stores fp16 (write traffic halves; l2 error ~1e-3, gate is 2e-2).
  * Sentinel formulation per guarded input x with per-channel (mid, rad):
      ScalarE:  q  = Square(x - mid)           (fp32, sign-exact compare)
                r' = Relu(HUGE*q - HUGE*rad^2) (bf16; 0 iff valid else >=1e22)
      Pool:     xb = tensor_copy(x)            (fp32 -> bf16 value copy)
      DVE:      s  = xb + r'                   (tensor_tensor add, bf16 2x)
                out= fmod(min(s, THR), THR)    (tensor_scalar fused, 4x;
                                                exact identity for |s| < THR,
                                                exactly 0 for sentinels)
    Vulnerable tiles build s_m and s_d this way and insert min(s_m, s_d)
    (bf16 tensor_tensor, 2x) before the fmod threshold.
  * Two simple tiles (A1, C1 -- the late ones) use the sentinel path; the
    other four use the plain two-pass scalar_tensor_tensor compare-mult
    (fused compare+apply, 1x) to keep ScalarE's 12-pass chain inside the
    DMA window.  Engine budget: DVE ~53us, ScalarE ~43us, Pool ~21us,
    PE unused, DMA 22.6MB @ ~430GB/s = ~53us.
  * bf16 value rounding (sentinel-path outputs) adds ~2e-3 l2; compare
    decisions always happen against fp32 inputs so no boundary flips
    beyond O(ulp) shifts of the parabola test.
"""

import os
import sys

for _p in ("/opt/trn_rl_repo", os.path.expanduser("~/.axon_site/_ro/trn_rl_repo")):
    if os.path.isdir(_p) and _p not in sys.path:
        sys.path.insert(0, _p)

import numpy as np

import concourse.bass as bass
import concourse.bacc as bacc
import concourse.mybir as mybir
from concourse.tile import TileContext
from concourse.bass_utils import run_bass_kernel_spmd

F32 = mybir.dt.float32
F16 = mybir.dt.float16
BF16 = mybir.dt.bfloat16
OP = mybir.AluOpType
AF = mybir.ActivationFunctionType

B, C, K, H, W = 32, 256, 64, 56, 56
HW = H * W
NCORES = 8
BL = B // NCORES  # batches per core

HUGE = 1.0e30  # sentinel prescale: HUGE * (q - rad^2) >> THR for any
               # practically-representable positive margin
THR = 1.0e15   # valid values are <= ~10; invalid sentinels are >= ~1e22

# bounds table columns (per-partition scalars for each tile kind)
#   0..3  : lo            for tile kinds A, B, C, V
#   4..7  : hi            for tile kinds A, B, C, V
#   8..11 : -mid          for tile kinds A, B, C, V  (mid = (lo+hi)/2)
#   12..15: -HUGE*rad^2   for tile kinds A, B, C, V  (rad = (hi-lo)/2)
NBCOLS = 16


def build_bounds(min_vals: np.ndarray, max_vals: np.ndarray) -> np.ndarray:
    lo = np.asarray(min_vals, dtype=np.float64)
    hi = np.asarray(max_vals, dtype=np.float64)
    cols = np.zeros((128, NBCOLS), dtype=np.float64)
    interleave = lambda a, b: np.stack([a, b], axis=1).ravel()
    kinds = [
        np.arange(64, 192),                                   # A: ch 64..191
        interleave(np.arange(192, 256), np.arange(64, 128)),  # B (interleaved)
        np.arange(128, 256),                                  # C: ch 128..255
        np.repeat(np.arange(0, 64), 2),                       # V (interleaved)
    ]
    for j, idx in enumerate(kinds):
        cols[:, j] = lo[idx]
        cols[:, 4 + j] = hi[idx]
        mid = (lo[idx] + hi[idx]) / 2.0
        rad = (hi[idx] - lo[idx]) / 2.0
        cols[:, 8 + j] = -mid
        cols[:, 12 + j] = -(HUGE * rad * rad)
    return cols.astype(np.float32)


def build_nc(hw: int = HW) -> bass.Bass:
    nc = bacc.Bacc("TRN2", target_bir_lowering=False, debug=False)
    R = BL * C
    main = nc.dram_tensor("main", [R, hw], F32, kind="ExternalInput")
    dup = nc.dram_tensor("dup", [BL * K, hw], F32, kind="ExternalInput")
    bounds = nc.dram_tensor("bounds", [128, NBCOLS], F32, kind="ExternalInput")
    out = nc.dram_tensor("out", [R, hw], F16, kind="ExternalOutput")

    npairs = BL // 2

    # Per-pair DRAM views.
    main_p = main.ap().rearrange("(p x) w -> p x w", p=npairs)   # [p, 512, hw]
    out_p = out.ap().rearrange("(p x) w -> p x w", p=npairs)
    dup_p = dup.ap().rearrange("(p s c) w -> p c s w", p=npairs, s=2)

    def v_ap(t):   # [64, 2, hw]: ch 0..63 of batches b, b+1 interleaved
        return t.rearrange("(s g c) w -> g c s w", s=2, g=4)[0]

    def b_ap(t):   # [64, 2, hw]: ch 192..255 of b / ch 64..127 of b+1
        return t[192:384].rearrange("(s c) w -> c s w", s=3)[:, 0:3:2]

    APS = {
        0: lambda t: t[64:192],      # A
        1: b_ap,                     # B
        2: lambda t: t[384:512],     # C
    }

    with TileContext(nc) as tc:
        with (
            tc.tile_pool(name="bnd", bufs=1) as bpool,
            tc.tile_pool(name="pm", bufs=6) as pm,
            tc.tile_pool(name="pv", bufs=2) as pv,
            tc.tile_pool(name="pd", bufs=2) as pd,
            tc.tile_pool(name="pq", bufs=1) as pq,
            tc.tile_pool(name="pr", bufs=4) as pr,
            tc.tile_pool(name="po", bufs=4) as po,
        ):
            bt = bpool.tile([128, NBCOLS], F32)
            nc.sync.dma_start(out=bt[:], in_=bounds[:])

            def lo_ap(j):
                return bt[:, j:j + 1]

            def hi_ap(j):
                return bt[:, 4 + j:5 + j]

            def nmid_ap(j):
                return bt[:, 8 + j:9 + j]

            def nhrad2_ap(j):
                return bt[:, 12 + j:13 + j]

            vd = [None] * npairs
            abc = [[None] * 3 for _ in range(npairs)]
            half = hw // 2

            def load_vd(p):
                mv = pv.tile([128, hw], F32, tag="mv")
                nc.sync.dma_start(out=mv[:], in_=v_ap(main_p[p]))
                dv = pd.tile([128, hw], F32, tag="dv")
                nc.sync.dma_start(out=dv[:], in_=dup_p[p])
                vd[p] = (mv, dv)

            def load_simple(p, kind, head=False):
                mt = pm.tile([128, hw], F32, tag="mt")
                abc[p][kind] = mt
                src_ap = APS[kind](main_p[p])
                if head:  # two half DMAs (returns the second so the caller
                    # can interleave other loads between them)
                    nc.sync.dma_start(out=mt[:, 0:half], in_=src_ap[..., 0:half])
                    return lambda: nc.sync.dma_start(
                        out=mt[:, half:hw], in_=src_ap[..., half:hw])
                nc.sync.dma_start(out=mt[:], in_=src_ap)
                return None

            # Load order: A0 halves first (DVE start), V0/D0 early (26us
            # ScalarE chain paces the vuln path), then B0, V1/D1, A1 before
            # C0/B1 (A1 is on the sentinel path: ScalarE needs it by ~34us).
            a0t = load_simple(0, 0, head=True)   # A0 first half
            mv0 = pv.tile([128, hw], F32, tag="mv", name="mv0")
            nc.sync.dma_start(out=mv0[:], in_=v_ap(main_p[0]))
            a0t()                                # A0 second half
            dv0 = pd.tile([128, hw], F32, tag="dv", name="dv0")
            nc.sync.dma_start(out=dv0[:], in_=dup_p[0])
            vd[0] = (mv0, dv0)
            load_simple(0, 1)                    # B0
            load_vd(1)                           # V1, D1
            load_simple(0, 2)                    # C0
            load_simple(1, 0)                    # A1 (sentinel path)
            load_simple(1, 1)                    # B1
            load_simple(1, 2)                    # C1 (sentinel path)

            # ScalarE compare chain: q = (x-mid)^2 in fp32 (sign-exact),
            # r' = relu(HUGE*q - HUGE*rad^2) in bf16 (0 iff valid, else
            # >= ~1e22).  Pool casts the value stream to bf16.
            def make_sent(x, j, name):
                q = pq.tile([128, hw], F32, tag="q", name=f"q{name}")
                nc.scalar.activation(q[:], x[:], AF.Square, bias=nmid_ap(j))
                r = pr.tile([128, hw], BF16, tag="rl", name=f"r{name}")
                nc.scalar.activation(r[:], q[:], AF.Relu, scale=HUGE,
                                     bias=nhrad2_ap(j))
                return r

            sent = []
            for p in range(npairs):
                mv, dv = vd[p]
                sent.append((make_sent(mv, 3, f"m{p}"),
                             make_sent(dv, 3, f"d{p}")))

            def thresh(ot, s, msk):
                # msk = (s < THR) in bf16 (exact 0/1), out = msk * s in fp16.
                # ts is 4x-capable on 16-bit SBUF operands, tt is 2x.
                nc.vector.tensor_scalar(out=msk[:], in0=s[:], scalar1=THR,
                                        scalar2=0.0, op0=OP.is_lt,
                                        op1=OP.bypass)
                nc.vector.tensor_tensor(out=ot[:], in0=msk[:], in1=s[:],
                                        op=OP.mult)

            def do_simple(p, kind, split=False):
                """Plain simple path on DVE: two fused compare-mult stt
                passes (1x), the second writing fp16."""
                mt = abc[p][kind]
                ot = po.tile([128, hw], F16, tag="ot")
                dst = APS[kind](out_p[p])
                stt = nc.vector.scalar_tensor_tensor
                halves = (slice(0, half), slice(half, hw)) if split \
                    else (slice(0, hw),)
                for cs in halves:
                    stt(out=mt[:, cs], in0=mt[:, cs], scalar=lo_ap(kind),
                        in1=mt[:, cs], op0=OP.is_ge, op1=OP.mult)
                    stt(out=ot[:, cs], in0=mt[:, cs], scalar=hi_ap(kind),
                        in1=mt[:, cs], op0=OP.is_le, op1=OP.mult)
                    nc.sync.dma_start(out=dst[..., cs], in_=ot[:, cs])

            def do_simple_sent(p, kind):
                """Sentinel-path simple tile: ScalarE does both compares,
                DVE applies in ONE fused stt: out = (r' == 0) * x."""
                mt = abc[p][kind]
                r = make_sent(mt, kind, f"s{p}{kind}")
                ot = po.tile([128, hw], F16, tag="ot")
                nc.vector.scalar_tensor_tensor(
                    out=ot[:], in0=r[:], scalar=0.0, in1=mt[:],
                    op0=OP.is_equal, op1=OP.mult)
                nc.sync.dma_start(out=APS[kind](out_p[p])[...], in_=ot[:])

            def do_vuln(p):
                """Vuln tile: s_m = bf16(m) + r'_m, s_d = bf16(d) + r'_d,
                u = min(s_m, s_d) (all bf16 tensor_tensor, 2x), then
                mask = (u < THR) (ts) and out = mask * u (tt)."""
                rm, rd = sent[p]
                mv, dv = vd[p]
                ot = po.tile([128, hw], F16, tag="ot", name=f"vot{p}")
                tt = nc.vector.tensor_tensor
                tt(out=rm[:], in0=mv[:], in1=rm[:], op=OP.add)
                tt(out=rd[:], in0=dv[:], in1=rd[:], op=OP.add)
                tt(out=rd[:], in0=rm[:], in1=rd[:], op=OP.min)
                thresh(ot, rd, rm)
                nc.sync.dma_start(out=v_ap(out_p[p])[...], in_=ot[:])

            # DVE queue order: plain tiles early (paced by loads), vuln and
            # sentinel tiles interleaved where their ScalarE/Pool inputs are
            # ready, sentinel applies last (cheap, ~3us each).
            do_simple(0, 0, split=True)     # A0
            do_simple(0, 1)                 # B0
            do_vuln(0)                      # V0
            do_simple(0, 2)                 # C0
            do_simple(1, 1)                 # B1
            do_simple_sent(1, 0)            # A1
            do_vuln(1)                      # V1
            do_simple_sent(1, 2)            # C1
    return nc


_NC_CACHE: dict = {}


def _get_nc(hw: int) -> bass.Bass:
    if hw not in _NC_CACHE:
        nc = build_nc(hw)
        nc.finalize()  # Bacc.finalize runs compile() (register allocation etc.)
        _NC_CACHE[hw] = nc
    return _NC_CACHE[hw]


def kernel(main_out, dup_out, min_vals, max_vals, vulnerable_idx):
    return _run(main_out, dup_out, min_vals, max_vals, vulnerable_idx)[0]


def _run(main_out, dup_out, min_vals, max_vals, vulnerable_idx, **spmd_kwargs):
    main_out = np.asarray(main_out)
    dup_out = np.asarray(dup_out)
    min_vals = np.asarray(min_vals)
    max_vals = np.asarray(max_vals)
    vidx = np.asarray(vulnerable_idx).ravel()

    # Device kernel assumes vulnerable channels are 0..K-1. If not, permute
    # channels host-side so they are, and invert on the way out.
    perm = None
    if not np.array_equal(vidx, np.arange(K)):
        assert len(np.unique(vidx)) == K, "duplicate vulnerable_idx unsupported"
        rest = np.setdiff1d(np.arange(C), vidx)
        perm = np.concatenate([vidx, rest])
        main_out = main_out[:, perm]
        min_vals = min_vals[perm]
        max_vals = max_vals[perm]

    mo = np.ascontiguousarray(main_out, dtype=np.float32).reshape(B, C, HW)
    du = np.ascontiguousarray(dup_out, dtype=np.float32).reshape(B, K, HW)
    bounds = build_bounds(min_vals, max_vals)

    in_maps = []
    for k in range(NCORES):
        in_maps.append({
            "main": mo[BL * k:BL * (k + 1)].reshape(BL * C, HW),
            "dup": du[BL * k:BL * (k + 1)].reshape(BL * K, HW),
            "bounds": bounds,
        })

    nc = _get_nc(HW)
    res = run_bass_kernel_spmd(nc, in_maps, list(range(NCORES)), **spmd_kwargs)
    out = np.concatenate(
        [r["out"].astype(np.float32).reshape(BL, C, H, W) for r in res.results],
        axis=0)

    if perm is not None:
        inv = np.empty(C, dtype=np.int64)
        inv[perm] = np.arange(C)
        out = out[:, inv]
    return out, res


# revision 13
# speedup vs baseline: 1.7044x; 1.0413x over previous
"""EDAC layer kernel for Trainium2 (8 NeuronCores, batch-sharded SPMD).

Reference semantics (B=32, C=256, K=64, H=W=56; vulnerable_idx == arange(K)):
  valid(x, c)  = min_vals[c] <= x <= max_vals[c]
  channels >= K:  out = x if valid else 0
  channels <  K:  m = main, d = dup
      both valid  -> min(m, d)      (covers m == d too)
      only d      -> d
      only m      -> m
      neither     -> 0

v3 design, driven by measured per-op DVE/ScalarE costs and the per-op perf
mode table (scalar_tensor_tensor: always 1x; tensor_tensor: 2x with all-bf16
operands; tensor_scalar(2 scalars): up to 4x; Pool: dtype-cast tensor_copy):

  * All stores fp16 (write traffic halves; l2 error ~1e-3, gate is 2e-2).
  * Sentinel formulation per guarded input x with per-channel (mid, rad):
      ScalarE:  q  = Square(x - mid)           (fp32, sign-exact compare)
                r' = Relu(HUGE*q - HUGE*rad^2) (bf16; 0 iff valid else >=1e22)
      Pool:     xb = tensor_copy(x)            (fp32 -> bf16 value copy)
      DVE:      s  = xb + r'                   (tensor_tensor add, bf16 2x)
                out= fmod(min(s, THR), THR)    (tensor_scalar fused, 4x;
                                                exact identity for |s| < THR,
                                                exactly 0 for sentinels)
    Vulnerable tiles build s_m and s_d this way and insert min(s_m, s_d)
    (bf16 tensor_tensor, 2x) before the fmod threshold.
  * Two simple tiles (A1, C1 -- the late ones) use the sentinel path; the
    other four use the plain two-pass scalar_tensor_tensor compare-mult
    (fused compare+apply, 1x) to keep ScalarE's 12-pass chain inside the
    DMA window.  Engine budget: DVE ~53us, ScalarE ~43us, Pool ~21us,
    PE unused, DMA 22.6MB @ ~430GB/s = ~53us.
  * bf16 value rounding (sentinel-path outputs) adds ~2e-3 l2; compare
    decisions always happen against fp32 inputs so no boundary flips
    beyond O(ulp) shifts of the parabola test.
"""

import os
import sys

for _p in ("/opt/trn_rl_repo", os.path.expanduser("~/.axon_site/_ro/trn_rl_repo")):
    if os.path.isdir(_p) and _p not in sys.path:
        sys.path.insert(0, _p)

import numpy as np

import concourse.bass as bass
import concourse.bacc as bacc
import concourse.mybir as mybir
from concourse.tile import TileContext
from concourse.bass_utils import run_bass_kernel_spmd

F32 = mybir.dt.float32
F16 = mybir.dt.float16
BF16 = mybir.dt.bfloat16
OP = mybir.AluOpType
AF = mybir.ActivationFunctionType

B, C, K, H, W = 32, 256, 64, 56, 56
HW = H * W
NCORES = 8
BL = B // NCORES  # batches per core

HUGE = 1.0e30  # sentinel prescale: HUGE * (q - rad^2) >> THR for any
               # practically-representable positive margin
THR = 1.0e15   # valid values are <= ~10; invalid sentinels are >= ~1e22

# bounds table columns (per-partition scalars for each tile kind)
#   0..3  : lo            for tile kinds A, B, C, V
#   4..7  : hi            for tile kinds A, B, C, V
#   8..11 : -mid          for tile kinds A, B, C, V  (mid = (lo+hi)/2)
#   12..15: -HUGE*rad^2   for tile kinds A, B, C, V  (rad = (hi-lo)/2)
NBCOLS = 16


def build_bounds(min_vals: np.ndarray, max_vals: np.ndarray) -> np.ndarray:
    lo = np.asarray(min_vals, dtype=np.float64)
    hi = np.asarray(max_vals, dtype=np.float64)
    cols = np.zeros((128, NBCOLS), dtype=np.float64)
    interleave = lambda a, b: np.stack([a, b], axis=1).ravel()
    kinds = [
        np.arange(64, 192),                                   # A: ch 64..191
        interleave(np.arange(192, 256), np.arange(64, 128)),  # B (interleaved)
        np.arange(128, 256),                                  # C: ch 128..255
        np.repeat(np.arange(0, 64), 2),                       # V (interleaved)
    ]
    for j, idx in enumerate(kinds):
        cols[:, j] = lo[idx]
        cols[:, 4 + j] = hi[idx]
        mid = (lo[idx] + hi[idx]) / 2.0
        rad = (hi[idx] - lo[idx]) / 2.0
        cols[:, 8 + j] = -mid
        cols[:, 12 + j] = -(HUGE * rad * rad)
    return cols.astype(np.float32)


def build_nc(hw: int = HW) -> bass.Bass:
    nc = bacc.Bacc("TRN2", target_bir_lowering=False, debug=False)
    R = BL * C
    main = nc.dram_tensor("main", [R, hw], F32, kind="ExternalInput")
    dup = nc.dram_tensor("dup", [BL * K, hw], F32, kind="ExternalInput")
    bounds = nc.dram_tensor("bounds", [128, NBCOLS], F32, kind="ExternalInput")
    out = nc.dram_tensor("out", [R, hw], F16, kind="ExternalOutput")

    npairs = BL // 2

    # Per-pair DRAM views.
    main_p = main.ap().rearrange("(p x) w -> p x w", p=npairs)   # [p, 512, hw]
    out_p = out.ap().rearrange("(p x) w -> p x w", p=npairs)
    dup_p = dup.ap().rearrange("(p s c) w -> p c s w", p=npairs, s=2)

    def v_ap(t):   # [64, 2, hw]: ch 0..63 of batches b, b+1 interleaved
        return t.rearrange("(s g c) w -> g c s w", s=2, g=4)[0]

    def b_ap(t):   # [64, 2, hw]: ch 192..255 of b / ch 64..127 of b+1
        return t[192:384].rearrange("(s c) w -> c s w", s=3)[:, 0:3:2]

    APS = {
        0: lambda t: t[64:192],      # A
        1: b_ap,                     # B
        2: lambda t: t[384:512],     # C
    }

    with TileContext(nc) as tc:
        with (
            tc.tile_pool(name="bnd", bufs=1) as bpool,
            tc.tile_pool(name="pm", bufs=6) as pm,
            tc.tile_pool(name="pv", bufs=2) as pv,
            tc.tile_pool(name="pd", bufs=2) as pd,
            tc.tile_pool(name="pq", bufs=1) as pq,
            tc.tile_pool(name="pr", bufs=4) as pr,
            tc.tile_pool(name="po", bufs=4) as po,
        ):
            bt = bpool.tile([128, NBCOLS], F32)
            nc.sync.dma_start(out=bt[:], in_=bounds[:])
            # 1-col dummy activation: forces the ScalarE ACT_TABLE_LOAD to
            # happen during the load phase instead of stalling the first
            # real compare pass.
            scr = bpool.tile([128, 1], F32, tag="scr")
            nc.scalar.activation(scr[:], bt[:, 0:1], AF.Square,
                                 bias=bt[:, 8:9])

            def lo_ap(j):
                return bt[:, j:j + 1]

            def hi_ap(j):
                return bt[:, 4 + j:5 + j]

            def nmid_ap(j):
                return bt[:, 8 + j:9 + j]

            def nhrad2_ap(j):
                return bt[:, 12 + j:13 + j]

            vd = [None] * npairs
            abc = [[None] * 3 for _ in range(npairs)]
            half = hw // 2

            def load_vd(p):
                mv = pv.tile([128, hw], F32, tag="mv")
                nc.sync.dma_start(out=mv[:], in_=v_ap(main_p[p]))
                dv = pd.tile([128, hw], F32, tag="dv")
                nc.sync.dma_start(out=dv[:], in_=dup_p[p])
                vd[p] = (mv, dv)

            def load_simple(p, kind, head=False):
                mt = pm.tile([128, hw], F32, tag="mt")
                abc[p][kind] = mt
                src_ap = APS[kind](main_p[p])
                if head:  # two half DMAs (returns the second so the caller
                    # can interleave other loads between them)
                    nc.sync.dma_start(out=mt[:, 0:half], in_=src_ap[..., 0:half])
                    return lambda: nc.sync.dma_start(
                        out=mt[:, half:hw], in_=src_ap[..., half:hw])
                nc.sync.dma_start(out=mt[:], in_=src_ap)
                return None

            # Load order: A0 halves first (DVE start), V0/D0 early (26us
            # ScalarE chain paces the vuln path), then B0, V1/D1, A1 before
            # C0/B1 (A1 is on the sentinel path: ScalarE needs it by ~34us).
            a0t = load_simple(0, 0, head=True)   # A0 first half
            mv0 = pv.tile([128, hw], F32, tag="mv", name="mv0")
            nc.sync.dma_start(out=mv0[:], in_=v_ap(main_p[0]))
            a0t()                                # A0 second half
            load_simple(0, 1)                    # B0
            dv0 = pd.tile([128, hw], F32, tag="dv", name="dv0")
            nc.sync.dma_start(out=dv0[:], in_=dup_p[0])
            vd[0] = (mv0, dv0)
            load_simple(0, 2)                    # C0 (sentinel path)
            load_vd(1)                           # V1, D1
            load_simple(1, 1)                    # B1
            load_simple(1, 0)                    # A1 (sentinel path)
            load_simple(1, 2)                    # C1 (sentinel path)

            # ScalarE compare chain: q = (x-mid)^2 in fp32 (sign-exact),
            # r' = relu(HUGE*q - HUGE*rad^2) in bf16 (0 iff valid, else
            # >= ~1e22).  Pool casts the value stream to bf16.
            def make_sent(x, j, name):
                q = pq.tile([128, hw], F32, tag="q", name=f"q{name}")
                nc.scalar.activation(q[:], x[:], AF.Square, bias=nmid_ap(j))
                r = pr.tile([128, hw], BF16, tag="rl", name=f"r{name}")
                nc.scalar.activation(r[:], q[:], AF.Relu, scale=HUGE,
                                     bias=nhrad2_ap(j))
                return r

            sent = []
            for p in range(npairs):
                mv, dv = vd[p]
                sent.append((make_sent(mv, 3, f"m{p}"),
                             make_sent(dv, 3, f"d{p}")))

            def thresh(ot, s, msk):
                # msk = (s < THR) in bf16 (exact 0/1), out = msk * s in fp16.
                # ts is 4x-capable on 16-bit SBUF operands, tt is 2x.
                nc.vector.tensor_scalar(out=msk[:], in0=s[:], scalar1=THR,
                                        scalar2=0.0, op0=OP.is_lt,
                                        op1=OP.bypass)
                nc.vector.tensor_tensor(out=ot[:], in0=msk[:], in1=s[:],
                                        op=OP.mult)

            def do_simple(p, kind, split=False):
                """Plain simple path on DVE: two fused compare-mult stt
                passes (1x), the second writing fp16."""
                mt = abc[p][kind]
                ot = po.tile([128, hw], F16, tag="ot")
                dst = APS[kind](out_p[p])
                stt = nc.vector.scalar_tensor_tensor
                halves = (slice(0, half), slice(half, hw)) if split \
                    else (slice(0, hw),)
                for cs in halves:
                    stt(out=mt[:, cs], in0=mt[:, cs], scalar=lo_ap(kind),
                        in1=mt[:, cs], op0=OP.is_ge, op1=OP.mult)
                    stt(out=ot[:, cs], in0=mt[:, cs], scalar=hi_ap(kind),
                        in1=mt[:, cs], op0=OP.is_le, op1=OP.mult)
                    nc.sync.dma_start(out=dst[..., cs], in_=ot[:, cs])

            def do_simple_sent(p, kind, split=False):
                """Sentinel-path simple tile: ScalarE does both compares,
                DVE applies in ONE fused stt: out = (r' == 0) * x."""
                mt = abc[p][kind]
                r = make_sent(mt, kind, f"s{p}{kind}")
                ot = po.tile([128, hw], F16, tag="ot")
                dst = APS[kind](out_p[p])
                halves = (slice(0, half), slice(half, hw)) if split \
                    else (slice(0, hw),)
                for cs in halves:
                    nc.vector.scalar_tensor_tensor(
                        out=ot[:, cs], in0=r[:, cs], scalar=0.0,
                        in1=mt[:, cs], op0=OP.is_equal, op1=OP.mult)
                    nc.sync.dma_start(out=dst[..., cs], in_=ot[:, cs])

            def do_vuln(p):
                """Vuln tile: s_m = bf16(m) + r'_m, s_d = bf16(d) + r'_d,
                u = min(s_m, s_d) (all bf16 tensor_tensor, 2x), then
                mask = (u < THR) (ts) and out = mask * u (tt)."""
                rm, rd = sent[p]
                mv, dv = vd[p]
                ot = po.tile([128, hw], F16, tag="ot", name=f"vot{p}")
                tt = nc.vector.tensor_tensor
                tt(out=rm[:], in0=mv[:], in1=rm[:], op=OP.add)
                tt(out=rd[:], in0=dv[:], in1=rd[:], op=OP.add)
                tt(out=rd[:], in0=rm[:], in1=rd[:], op=OP.min)
                thresh(ot, rd, rm)
                nc.sync.dma_start(out=v_ap(out_p[p])[...], in_=ot[:])

            # DVE queue order: plain tiles early (paced by loads), vuln and
            # sentinel tiles interleaved where their ScalarE/Pool inputs are
            # ready, sentinel applies last (cheap, ~3us each).
            do_simple(0, 0, split=True)     # A0
            do_simple(0, 1)                 # B0
            do_vuln(0)                      # V0
            do_simple_sent(0, 2)            # C0
            do_simple(1, 1)                 # B1
            do_vuln(1)                      # V1
            do_simple_sent(1, 0)            # A1
            do_simple_sent(1, 2, split=True)  # C1
    return nc


_NC_CACHE: dict = {}


def _get_nc(hw: int) -> bass.Bass:
    if hw not in _NC_CACHE:
        nc = build_nc(hw)
        nc.finalize()  # Bacc.finalize runs compile() (register allocation etc.)
        _NC_CACHE[hw] = nc
    return _NC_CACHE[hw]


def kernel(main_out, dup_out, min_vals, max_vals, vulnerable_idx):
    return _run(main_out, dup_out, min_vals, max_vals, vulnerable_idx)[0]


def _run(main_out, dup_out, min_vals, max_vals, vulnerable_idx, **spmd_kwargs):
    main_out = np.asarray(main_out)
    dup_out = np.asarray(dup_out)
    min_vals = np.asarray(min_vals)
    max_vals = np.asarray(max_vals)
    vidx = np.asarray(vulnerable_idx).ravel()

    # Device kernel assumes vulnerable channels are 0..K-1. If not, permute
    # channels host-side so they are, and invert on the way out.
    perm = None
    if not np.array_equal(vidx, np.arange(K)):
        assert len(np.unique(vidx)) == K, "duplicate vulnerable_idx unsupported"
        rest = np.setdiff1d(np.arange(C), vidx)
        perm = np.concatenate([vidx, rest])
        main_out = main_out[:, perm]
        min_vals = min_vals[perm]
        max_vals = max_vals[perm]

    mo = np.ascontiguousarray(main_out, dtype=np.float32).reshape(B, C, HW)
    du = np.ascontiguousarray(dup_out, dtype=np.float32).reshape(B, K, HW)
    bounds = build_bounds(min_vals, max_vals)

    in_maps = []
    for k in range(NCORES):
        in_maps.append({
            "main": mo[BL * k:BL * (k + 1)].reshape(BL * C, HW),
            "dup": du[BL * k:BL * (k + 1)].reshape(BL * K, HW),
            "bounds": bounds,
        })

    nc = _get_nc(HW)
    res = run_bass_kernel_spmd(nc, in_maps, list(range(NCORES)), **spmd_kwargs)
    out = np.concatenate(
        [r["out"].astype(np.float32).reshape(BL, C, H, W) for r in res.results],
        axis=0)

    if perm is not None:
        inv = np.empty(C, dtype=np.int64)
        inv[perm] = np.arange(C)
        out = out[:, inv]
    return out, res


# revision 14
# speedup vs baseline: 1.7798x; 1.0443x over previous
"""EDAC layer kernel for Trainium2 (8 NeuronCores, batch-sharded SPMD).

Reference semantics (B=32, C=256, K=64, H=W=56; vulnerable_idx == arange(K)):
  valid(x, c)  = min_vals[c] <= x <= max_vals[c]
  channels >= K:  out = x if valid else 0
  channels <  K:  m = main, d = dup
      both valid  -> min(m, d)      (covers m == d too)
      only d      -> d
      only m      -> m
      neither     -> 0

v3 design, driven by measured per-op DVE/ScalarE costs and the per-op perf
mode table (scalar_tensor_tensor: always 1x; tensor_tensor: 2x with all-bf16
operands; tensor_scalar(2 scalars): up to 4x; Pool: dtype-cast tensor_copy):

  * All stores fp16 (write traffic halves; l2 error ~1e-3, gate is 2e-2).
  * Sentinel formulation per guarded input x with per-channel (mid, rad):
      ScalarE:  q  = Square(x - mid)           (fp32, sign-exact compare)
                r' = Relu(HUGE*q - HUGE*rad^2) (bf16; 0 iff valid else >=1e22)
      Pool:     xb = tensor_copy(x)            (fp32 -> bf16 value copy)
      DVE:      s  = xb + r'                   (tensor_tensor add, bf16 2x)
                out= fmod(min(s, THR), THR)    (tensor_scalar fused, 4x;
                                                exact identity for |s| < THR,
                                                exactly 0 for sentinels)
    Vulnerable tiles build s_m and s_d this way and insert min(s_m, s_d)
    (bf16 tensor_tensor, 2x) before the fmod threshold.
  * Two simple tiles (A1, C1 -- the late ones) use the sentinel path; the
    other four use the plain two-pass scalar_tensor_tensor compare-mult
    (fused compare+apply, 1x) to keep ScalarE's 12-pass chain inside the
    DMA window.  Engine budget: DVE ~53us, ScalarE ~43us, Pool ~21us,
    PE unused, DMA 22.6MB @ ~430GB/s = ~53us.
  * bf16 value rounding (sentinel-path outputs) adds ~2e-3 l2; compare
    decisions always happen against fp32 inputs so no boundary flips
    beyond O(ulp) shifts of the parabola test.
"""

import os
import sys

for _p in ("/opt/trn_rl_repo", os.path.expanduser("~/.axon_site/_ro/trn_rl_repo")):
    if os.path.isdir(_p) and _p not in sys.path:
        sys.path.insert(0, _p)

import numpy as np

import concourse.bass as bass
import concourse.bacc as bacc
import concourse.mybir as mybir
from concourse.tile import TileContext
from concourse.bass_utils import run_bass_kernel_spmd

F32 = mybir.dt.float32
F16 = mybir.dt.float16
BF16 = mybir.dt.bfloat16
OP = mybir.AluOpType
AF = mybir.ActivationFunctionType

B, C, K, H, W = 32, 256, 64, 56, 56
HW = H * W
NCORES = 8
BL = B // NCORES  # batches per core

HUGE = 1.0e30  # sentinel prescale: HUGE * (q - rad^2) >> THR for any
               # practically-representable positive margin
THR = 1.0e15   # valid values are <= ~10; invalid sentinels are >= ~1e22

# bounds table columns (per-partition scalars for each tile kind)
#   0..3  : lo            for tile kinds A, B, C, V
#   4..7  : hi            for tile kinds A, B, C, V
#   8..11 : -mid          for tile kinds A, B, C, V  (mid = (lo+hi)/2)
#   12..15: -HUGE*rad^2   for tile kinds A, B, C, V  (rad = (hi-lo)/2)
NBCOLS = 16


def build_bounds(min_vals: np.ndarray, max_vals: np.ndarray) -> np.ndarray:
    lo = np.asarray(min_vals, dtype=np.float64)
    hi = np.asarray(max_vals, dtype=np.float64)
    cols = np.zeros((128, NBCOLS), dtype=np.float64)
    interleave = lambda a, b: np.stack([a, b], axis=1).ravel()
    kinds = [
        np.arange(64, 192),                                   # A: ch 64..191
        interleave(np.arange(192, 256), np.arange(64, 128)),  # B (interleaved)
        np.arange(128, 256),                                  # C: ch 128..255
        np.repeat(np.arange(0, 64), 2),                       # V (interleaved)
    ]
    for j, idx in enumerate(kinds):
        cols[:, j] = lo[idx]
        cols[:, 4 + j] = hi[idx]
        mid = (lo[idx] + hi[idx]) / 2.0
        rad = (hi[idx] - lo[idx]) / 2.0
        cols[:, 8 + j] = -mid
        cols[:, 12 + j] = -(HUGE * rad * rad)
    return cols.astype(np.float32)


def build_nc(hw: int = HW) -> bass.Bass:
    nc = bacc.Bacc("TRN2", target_bir_lowering=False, debug=False)
    R = BL * C
    main = nc.dram_tensor("main", [R, hw], F32, kind="ExternalInput")
    dup = nc.dram_tensor("dup", [BL * K, hw], F32, kind="ExternalInput")
    bounds = nc.dram_tensor("bounds", [128, NBCOLS], F32, kind="ExternalInput")
    out = nc.dram_tensor("out", [R, hw], F16, kind="ExternalOutput")

    npairs = BL // 2

    # Per-pair DRAM views.
    main_p = main.ap().rearrange("(p x) w -> p x w", p=npairs)   # [p, 512, hw]
    out_p = out.ap().rearrange("(p x) w -> p x w", p=npairs)
    dup_p = dup.ap().rearrange("(p s c) w -> p c s w", p=npairs, s=2)

    def v_ap(t):   # [64, 2, hw]: ch 0..63 of batches b, b+1 interleaved
        return t.rearrange("(s g c) w -> g c s w", s=2, g=4)[0]

    def b_ap(t):   # [64, 2, hw]: ch 192..255 of b / ch 64..127 of b+1
        return t[192:384].rearrange("(s c) w -> c s w", s=3)[:, 0:3:2]

    APS = {
        0: lambda t: t[64:192],      # A
        1: b_ap,                     # B
        2: lambda t: t[384:512],     # C
    }

    with TileContext(nc) as tc:
        with (
            tc.tile_pool(name="bnd", bufs=1) as bpool,
            tc.tile_pool(name="pm", bufs=6) as pm,
            tc.tile_pool(name="pv", bufs=2) as pv,
            tc.tile_pool(name="pd", bufs=2) as pd,
            tc.tile_pool(name="pq", bufs=1) as pq,
            tc.tile_pool(name="pr", bufs=4) as pr,
            tc.tile_pool(name="po", bufs=4) as po,
        ):
            bt = bpool.tile([128, NBCOLS], F32)
            nc.sync.dma_start(out=bt[:], in_=bounds[:])
            # 1-col dummy activation: forces the ScalarE ACT_TABLE_LOAD to
            # happen during the load phase instead of stalling the first
            # real compare pass.
            scr = bpool.tile([128, 1], F32, tag="scr")
            nc.scalar.activation(scr[:], bt[:, 0:1], AF.Square,
                                 bias=bt[:, 8:9])

            def lo_ap(j):
                return bt[:, j:j + 1]

            def hi_ap(j):
                return bt[:, 4 + j:5 + j]

            def nmid_ap(j):
                return bt[:, 8 + j:9 + j]

            def nhrad2_ap(j):
                return bt[:, 12 + j:13 + j]

            vd = [None] * npairs
            abc = [[None] * 3 for _ in range(npairs)]
            half = hw // 2

            def load_vd(p):
                mv = pv.tile([128, hw], F32, tag="mv")
                nc.sync.dma_start(out=mv[:], in_=v_ap(main_p[p]))
                dv = pd.tile([128, hw], F32, tag="dv")
                nc.sync.dma_start(out=dv[:], in_=dup_p[p])
                vd[p] = (mv, dv)

            def load_simple(p, kind, head=False):
                mt = pm.tile([128, hw], F32, tag="mt")
                abc[p][kind] = mt
                src_ap = APS[kind](main_p[p])
                if head:  # two half DMAs (returns the second so the caller
                    # can interleave other loads between them)
                    nc.sync.dma_start(out=mt[:, 0:half], in_=src_ap[..., 0:half])
                    return lambda: nc.sync.dma_start(
                        out=mt[:, half:hw], in_=src_ap[..., half:hw])
                nc.sync.dma_start(out=mt[:], in_=src_ap)
                return None

            # Load order: A0 halves first (DVE start), V0/D0 early (26us
            # ScalarE chain paces the vuln path), then B0, V1/D1, A1 before
            # C0/B1 (A1 is on the sentinel path: ScalarE needs it by ~34us).
            a0t = load_simple(0, 0, head=True)   # A0 first half
            a0t()                                # A0 second half
            mv0 = pv.tile([128, hw], F32, tag="mv", name="mv0")
            nc.sync.dma_start(out=mv0[:], in_=v_ap(main_p[0]))
            b0t = load_simple(0, 1, head=True)   # B0 first half
            b0t()                                # B0 second half
            dv0 = pd.tile([128, hw], F32, tag="dv", name="dv0")
            nc.sync.dma_start(out=dv0[:], in_=dup_p[0])
            vd[0] = (mv0, dv0)
            load_simple(0, 2)                    # C0 (sentinel path)
            load_vd(1)                           # V1, D1
            load_simple(1, 1)                    # B1
            load_simple(1, 0)                    # A1 (sentinel path)
            load_simple(1, 2)                    # C1 (sentinel path)

            # ScalarE compare chain: q = (x-mid)^2 in fp32 (sign-exact),
            # r' = relu(HUGE*q - HUGE*rad^2) in bf16 (0 iff valid, else
            # >= ~1e22).  Pool casts the value stream to bf16.
            def make_sent(x, j, name):
                q = pq.tile([128, hw], F32, tag="q", name=f"q{name}")
                nc.scalar.activation(q[:], x[:], AF.Square, bias=nmid_ap(j))
                r = pr.tile([128, hw], BF16, tag="rl", name=f"r{name}")
                nc.scalar.activation(r[:], q[:], AF.Relu, scale=HUGE,
                                     bias=nhrad2_ap(j))
                return r

            sent = []
            for p in range(npairs):
                mv, dv = vd[p]
                sent.append((make_sent(mv, 3, f"m{p}"),
                             make_sent(dv, 3, f"d{p}")))

            def thresh(ot, s, msk):
                # msk = (s < THR) in bf16 (exact 0/1), out = msk * s in fp16.
                # ts is 4x-capable on 16-bit SBUF operands, tt is 2x.
                nc.vector.tensor_scalar(out=msk[:], in0=s[:], scalar1=THR,
                                        scalar2=0.0, op0=OP.is_lt,
                                        op1=OP.bypass)
                nc.vector.tensor_tensor(out=ot[:], in0=msk[:], in1=s[:],
                                        op=OP.mult)

            def do_simple(p, kind, split=False):
                """Plain simple path on DVE: two fused compare-mult stt
                passes (1x), the second writing fp16."""
                mt = abc[p][kind]
                ot = po.tile([128, hw], F16, tag="ot")
                dst = APS[kind](out_p[p])
                stt = nc.vector.scalar_tensor_tensor
                halves = (slice(0, half), slice(half, hw)) if split \
                    else (slice(0, hw),)
                for cs in halves:
                    stt(out=mt[:, cs], in0=mt[:, cs], scalar=lo_ap(kind),
                        in1=mt[:, cs], op0=OP.is_ge, op1=OP.mult)
                    stt(out=ot[:, cs], in0=mt[:, cs], scalar=hi_ap(kind),
                        in1=mt[:, cs], op0=OP.is_le, op1=OP.mult)
                    nc.sync.dma_start(out=dst[..., cs], in_=ot[:, cs])

            def do_simple_sent(p, kind, split=False):
                """Sentinel-path simple tile: ScalarE does both compares,
                DVE applies in ONE fused stt: out = (r' == 0) * x."""
                mt = abc[p][kind]
                r = make_sent(mt, kind, f"s{p}{kind}")
                ot = po.tile([128, hw], F16, tag="ot")
                dst = APS[kind](out_p[p])
                halves = (slice(0, half), slice(half, hw)) if split \
                    else (slice(0, hw),)
                for cs in halves:
                    nc.vector.scalar_tensor_tensor(
                        out=ot[:, cs], in0=r[:, cs], scalar=0.0,
                        in1=mt[:, cs], op0=OP.is_equal, op1=OP.mult)
                    nc.sync.dma_start(out=dst[..., cs], in_=ot[:, cs])

            def do_vuln(p):
                """Vuln tile: s_m = bf16(m) + r'_m, s_d = bf16(d) + r'_d,
                u = min(s_m, s_d) (all bf16 tensor_tensor, 2x), then
                mask = (u < THR) (ts) and out = mask * u (tt)."""
                rm, rd = sent[p]
                mv, dv = vd[p]
                ot = po.tile([128, hw], F16, tag="ot", name=f"vot{p}")
                tt = nc.vector.tensor_tensor
                tt(out=rm[:], in0=mv[:], in1=rm[:], op=OP.add)
                tt(out=rd[:], in0=dv[:], in1=rd[:], op=OP.add)
                tt(out=rd[:], in0=rm[:], in1=rd[:], op=OP.min)
                thresh(ot, rd, rm)
                nc.sync.dma_start(out=v_ap(out_p[p])[...], in_=ot[:])

            # DVE queue order: plain tiles early (paced by loads), vuln and
            # sentinel tiles interleaved where their ScalarE/Pool inputs are
            # ready, sentinel applies last (cheap, ~3us each).
            do_simple(0, 0, split=True)     # A0
            do_simple(0, 1, split=True)     # B0
            do_vuln(0)                      # V0
            do_simple_sent(0, 2)            # C0
            do_simple(1, 1)                 # B1
            do_vuln(1)                      # V1
            do_simple_sent(1, 0)            # A1
            do_simple_sent(1, 2, split=True)  # C1
    return nc


_NC_CACHE: dict = {}


def _get_nc(hw: int) -> bass.Bass:
    if hw not in _NC_CACHE:
        nc = build_nc(hw)
        nc.finalize()  # Bacc.finalize runs compile() (register allocation etc.)
        _NC_CACHE[hw] = nc
    return _NC_CACHE[hw]


def kernel(main_out, dup_out, min_vals, max_vals, vulnerable_idx):
    return _run(main_out, dup_out, min_vals, max_vals, vulnerable_idx)[0]


def _run(main_out, dup_out, min_vals, max_vals, vulnerable_idx, **spmd_kwargs):
    main_out = np.asarray(main_out)
    dup_out = np.asarray(dup_out)
    min_vals = np.asarray(min_vals)
    max_vals = np.asarray(max_vals)
    vidx = np.asarray(vulnerable_idx).ravel()

    # Device kernel assumes vulnerable channels are 0..K-1. If not, permute
    # channels host-side so they are, and invert on the way out.
    perm = None
    if not np.array_equal(vidx, np.arange(K)):
        assert len(np.unique(vidx)) == K, "duplicate vulnerable_idx unsupported"
        rest = np.setdiff1d(np.arange(C), vidx)
        perm = np.concatenate([vidx, rest])
        main_out = main_out[:, perm]
        min_vals = min_vals[perm]
        max_vals = max_vals[perm]

    mo = np.ascontiguousarray(main_out, dtype=np.float32).reshape(B, C, HW)
    du = np.ascontiguousarray(dup_out, dtype=np.float32).reshape(B, K, HW)
    bounds = build_bounds(min_vals, max_vals)

    in_maps = []
    for k in range(NCORES):
        in_maps.append({
            "main": mo[BL * k:BL * (k + 1)].reshape(BL * C, HW),
            "dup": du[BL * k:BL * (k + 1)].reshape(BL * K, HW),
            "bounds": bounds,
        })

    nc = _get_nc(HW)
    res = run_bass_kernel_spmd(nc, in_maps, list(range(NCORES)), **spmd_kwargs)
    out = np.concatenate(
        [r["out"].astype(np.float32).reshape(BL, C, H, W) for r in res.results],
        axis=0)

    if perm is not None:
        inv = np.empty(C, dtype=np.int64)
        inv[perm] = np.arange(C)
        out = out[:, inv]
    return out, res
